# revision 22
# baseline (speedup 1.0000x reference)
"""Trainium2 Bass kernel for the spectral history-filter model (nn_DSC_23252952941334).

Math: all four reference terms are linear in y_hist with per-lag effective
weight matrices, so the whole module collapses to

    out[b, c] = sum_{j=0..63} sum_p  W_time[j][c, p] * y_hist[b, j+2, p]

where W_time[j] combines M_bar_0 / M_bar / M_0l / M_il with the small
spectral coefficient vectors (folded on host in float64 — ~5 MFLOP).

Device strategy (8 cores, data-parallel over batch), mode "f8a":
  - host: fold weights (f16, scaled 1/ALPHA), quantize each core's y
    shard to fp8e3 (E3M4) * ALPHA in (P, NLAG, BS) layout.  Output
    rel-err from the fp8 y is 1.33e-2 on the real data (gate 2e-2);
    the PE computes mixed-dtype matmuls (f16 stationary x fp8e3 moving,
    HW-verified exact) at 1 col/cycle, and fp8 halves the binding HBM
    traffic to ~19.3 MB/core.
  - device: out[c, b] = sum_j w_j[c, p] @ y_j[p, b] as 64 accumulated
    128-contraction matmuls into 4 rotating PSUM banks (b free 512 each;
    consecutive matmuls must hit different banks — same-bank chaining
    measured ~18% slower).  w arrives in 8 chunks whose waits ride the
    per-matmul LDWEIGHTS; first y chunks are small (2 lags) so the MM
    stream starts ~5.5 us after the framework preamble.  PSUM evacuation
    split DVE/ACT, f16 output, two half out-DMAs.
  - host: gather per-core [c, b] f16 outputs, transpose to (B, 128) f32.
Measured: 77.6 us HW exec (vs 111.5 us f16 baseline), rel err 1.329e-2.
"""

import os
import numpy as np
from concurrent.futures import ThreadPoolExecutor

N_CORES = 8
B, L, P, MC = 16384, 66, 128, 128
H, M = 16, 32
NLAG = 64            # lags d=0..63 <-> y time indices 2..65
BS = B // N_CORES    # 2048 batch rows per core
NBT = 4              # psum b-tiles of 512
KJ = None            # k-chunks per DMA super-chunk; set per mode below

# Data dtype for y/w on device.  The harness gate is rel_err < 2e-2;
# measured accuracy on real data: f16 2.5e-4, f32r 1.25e-4, f32 2.6e-7,
# fp8e3 y + f16 w 1.33e-2 (quant_study.py).
# f16 halves HBM traffic (the binding roofline): ~100us vs ~192us per core.
# fp8 y halves it again (~50us) and PE at 1 cyc/row needs ~55us -> ~60us.
#   "f16"  -> float16 tiles (host casts)
#   "f32r" -> fp32 data, float32r matmuls (1 cyc/row at free>=256)
#   "f32"  -> plain fp32 matmuls (4 cyc/row, PE-bound)
#   "f8a"  -> y fp8e3 moving, w f16 stationary (mixed-dtype matmul)
#   "f8b"  -> y fp8e3 over DMA, DVE casts to f16 on-chip, f16 matmuls
MODE = os.environ.get("KERNEL_MODE", "f8a")
ALPHA = 2.5          # y scale before fp8e3 encode; w carries 1/ALPHA
TRACE = False        # test.py can flip this to get a profile

_cached_nc = {}


def _fold_weights(M_bar_0, M_bar, M_0l, M_il, sigma_powered, phi,
                  lambda_powered, varphi):
    """Return w_dev (P, NLAG, MC) fp32 with w_dev[p, j, c] = W_time[j][c, p]."""
    f8 = np.float64
    M_bar_0 = M_bar_0.astype(f8); M_bar = M_bar.astype(f8)
    M_0l = M_0l.astype(f8); M_il = M_il.astype(f8)
    sig = sigma_powered.astype(f8); lam = lambda_powered.astype(f8)
    phi = phi.astype(f8); varphi = varphi.astype(f8)

    # W_lag[d] acts on Yr[:, d] = y[:, L-1-d]
    W = np.zeros((NLAG, MC, P), f8)
    W[0] = M_bar_0

    # term 2: sum_i lam[i] * varphi[j, i] * M_bar[i] on lag j+1
    coef2 = varphi @ np.diag(lam)                       # (M, H) -> [j, i]
    W[1:M + 1] += np.einsum('ji,icp->jcp', coef2, M_bar)

    # term 3: sum_l sigma_ext[l] * phi_ext[k, l] * M_0l[l] on lag k+1
    sigma_ext = np.concatenate([[1.0], sig])            # (H+1,)
    phi_ext = np.concatenate([np.ones((M, 1)), phi], 1)  # (M, H+1)
    coef3 = phi_ext @ np.diag(sigma_ext)                # (M, H+1) -> [k, l]
    W[1:M + 1] += np.einsum('kl,lcp->kcp', coef3, M_0l)

    # term 4: anti-diagonal fold of varphi[j,i] phi_ext[k,l] comb[l,i] M_il[i,l]
    comb = sigma_ext[:, None] * lam[None, :]            # (H+1, H) -> [l, i]
    corr = np.zeros((2 * M - 1, H + 1, H), f8)          # [d, l, i]
    for j in range(M):
        for k in range(M):
            corr[j + k] += phi_ext[k][:, None] * varphi[j][None, :]
    C4 = corr * comb[None]                              # (2M-1, H+1, H)
    W[1:2 * M] += np.einsum('dli,ilcp->dcp', C4, M_il)

    # reorder to ascending time index: W_time[j] = W_lag[63 - j]
    # and lay out for SBUF: w_dev[p, j, c]
    w_dev = np.ascontiguousarray(W[::-1].transpose(2, 0, 1)).astype(np.float32)
    return w_dev


def _transpose_shards(y, npdt, scale=None):
    """y (B, L, P) fp32 -> list of per-core yt (NLAG, P, BS) npdt,
    yt[j, p, b] = scale * y[core*BS + b, j + 2, p]."""
    src = y[:, 2:, :]                  # (B, 64, 128) strided view
    shards = [np.empty((NLAG, P, BS), npdt) for _ in range(N_CORES)]
    BB = 128

    def work(args):
        ci, b0 = args
        blk = np.ascontiguousarray(src[ci * BS + b0: ci * BS + b0 + BB])
        if scale is not None:
            blk = blk * np.float32(scale)
        if npdt != np.float32:
            blk = blk.astype(npdt)
        shards[ci][:, :, b0:b0 + BB] = blk.transpose(1, 2, 0)

    jobs = [(ci, b0) for ci in range(N_CORES) for b0 in range(0, BS, BB)]
    with ThreadPoolExecutor(8) as ex:
        list(ex.map(work, jobs))
    return shards


def _transpose_shards_pjb(y, npdt, scale=None):
    """y (B, L, P) fp32 -> list of per-core yt (P, NLAG, BS) npdt,
    yt[p, j, b] = scale * y[core*BS + b, j + 2, p].
    Per-partition (j, b) planes are contiguous -> 2 KiB+ DMA descriptors."""
    src = y[:, 2:, :]                  # (B, 64, 128) strided view
    shards = [np.empty((P, NLAG, BS), npdt) for _ in range(N_CORES)]
    BB = 256

    def work(args):
        ci, b0 = args
        blk = np.ascontiguousarray(src[ci * BS + b0: ci * BS + b0 + BB])
        if scale is not None:
            blk = blk * np.float32(scale)
        blk = np.ascontiguousarray(blk.transpose(2, 1, 0))  # (P, NLAG, BB)
        if npdt != np.float32:
            blk = blk.astype(npdt)
        shards[ci][:, :, b0:b0 + BB] = blk

    jobs = [(ci, b0) for ci in range(N_CORES) for b0 in range(0, BS, BB)]
    with ThreadPoolExecutor(8) as ex:
        list(ex.map(work, jobs))
    return shards


def _mode_params(mode):
    import ml_dtypes
    from concourse import mybir
    # npdt/ddt/rdt: y dtype (numpy / dram-declared / matmul-view);
    # wnpdt/wdt: w dtype; yscale: host multiplies y, w carries 1/yscale.
    if mode == "f16":
        return dict(npdt=np.float16, ddt=mybir.dt.float16,
                    rdt=mybir.dt.float16, wnpdt=np.float16,
                    wdt=mybir.dt.float16, yscale=None, kj=4, ybufs=8)
    if mode == "f32r":
        return dict(npdt=np.float32, ddt=mybir.dt.float32,
                    rdt=mybir.dt.float32r, wnpdt=np.float32,
                    wdt=mybir.dt.float32, yscale=None, kj=2, ybufs=8)
    if mode == "f32":
        return dict(npdt=np.float32, ddt=mybir.dt.float32,
                    rdt=mybir.dt.float32, wnpdt=np.float32,
                    wdt=mybir.dt.float32, yscale=None, kj=2, ybufs=8)
    if mode == "f8a":
        return dict(npdt=ml_dtypes.float8_e3m4, ddt=mybir.dt.float8e3,
                    rdt=mybir.dt.float8e3, wnpdt=np.float16,
                    wdt=mybir.dt.float16, yscale=ALPHA, kj=4, ybufs=8)
    raise ValueError(mode)


def _build_nc(mode):
    import concourse.bass as bass
    import concourse.tile as tile
    from concourse import mybir
    from concourse.bass import ts
    from contextlib import ExitStack

    mp = _mode_params(mode)
    DT = mybir.dt.float32
    DDT = mp["ddt"]
    RDT = mp["rdt"]
    WDT = mp["wdt"]
    WRDT = mybir.dt.float32r if mode == "f32r" else WDT
    KJ = mp["kj"]

    nc = bass.Bass()
    yt = nc.declare_dram_parameter("yt", [NLAG, P, BS], DDT, isOutput=False)
    w = nc.declare_dram_parameter("w", [P, NLAG, MC], WDT, isOutput=False)
    out = nc.declare_dram_parameter("out", [MC, BS], DT, isOutput=True)

    NSUP = NLAG // KJ  # DMA super-chunks

    with ExitStack() as ctx:
        tc = ctx.enter_context(tile.TileContext(nc))
        wpool = ctx.enter_context(tc.tile_pool(name="wp", bufs=1))
        ypool = ctx.enter_context(
            tc.tile_pool(name="yp", bufs=mp["ybufs"]))
        opool = ctx.enter_context(tc.tile_pool(name="op", bufs=1))
        pspool = ctx.enter_context(tc.tile_pool(name="ps", bufs=1, space="PSUM"))

        wtile = wpool.tile([P, NLAG, MC], WRDT)
        nc.sync.dma_start(wtile[:], w[:].bitcast(WRDT))

        psums = [pspool.tile([MC, 512], mybir.dt.float32, name=f"ps{t}")
                 for t in range(NBT)]

        # Warm-up matmuls consuming wtile: give the PE a single-wait
        # observation of the w-DMA (walrus rejects >1 sync wait on a
        # matmul) and ramp the HAM clock gate to 8/8 before the real
        # work (~2us of dense PE activity during the first y transfer).
        warm = pspool.tile([MC, 512], mybir.dt.float32, name="warm")
        for wi in range(8):
            nc.tensor.matmul(warm[:], wtile[:, wi, :],
                             wtile[:, 8 * wi:8 * wi + 4, :],
                             start=(wi == 0), stop=(wi == 7))

        for kk in range(NSUP):
            ytile = ypool.tile([P, KJ, BS], RDT)
            src = yt[kk * KJ:(kk + 1) * KJ, :, :].rearrange("j p b -> p j b")
            nc.sync.dma_start(ytile[:], src.bitcast(RDT))
            for jj in range(KJ):
                j = kk * KJ + jj
                lhsT = wtile[:, j, :]
                for t in range(NBT):
                    rhs = ytile[:, jj, ts(t, 512)]
                    nc.tensor.matmul(psums[t][:], lhsT, rhs,
                                     start=(j == 0), stop=(j == NLAG - 1))

        outt = opool.tile([MC, BS], DT)
        for t in range(NBT):
            nc.vector.tensor_copy(outt[:, ts(t, 512)], psums[t][:])
        nc.sync.dma_start(out[:], outt[:])

    return nc


def _build_nc_f8():
    """fp8e3 y (moving) x f16 w (stationary) mixed-dtype matmuls.

    Layout: yt (P, NLAG, BS) so each lag-chunk DMA reads contiguous 8 KiB
    per partition.  Lag-outer loop with the psum-bank rotation of the f16
    baseline (consecutive matmuls target different banks — same-bank
    back-to-back accumulation measured ~18% slower per matmul).  w arrives
    in 8 chunks whose waits ride the per-matmul LDWEIGHTS, so the PE
    starts after w-chunk 0 + y-chunk 0 (~2 MB earlier than whole-w).
    Tail: PSUM evacuation split DVE/ACT, two half out-DMAs (each a single
    sync wait), f16 output.
    """
    import concourse.bass as bass
    import concourse.tile as tile
    from concourse import mybir
    from concourse.bass import ts
    from contextlib import ExitStack

    F16 = mybir.dt.float16
    F8 = mybir.dt.float8e3
    F32 = mybir.dt.float32
    KJ = 4                     # lags per y chunk
    NSUP = NLAG // KJ          # y chunks
    NB = BS // NBT             # 512 columns per psum tile
    WCH = 8                    # w chunks (8 lags each)
    WL = NLAG // WCH

    nc = bass.Bass()
    yt = nc.declare_dram_parameter("yt", [P, NLAG, BS], F8, isOutput=False)
    w = nc.declare_dram_parameter("w", [P, NLAG, MC], F16, isOutput=False)
    out = nc.declare_dram_parameter("out", [MC, BS], F16, isOutput=True)

    with ExitStack() as ctx:
        tc = ctx.enter_context(tile.TileContext(nc))
        wpool = ctx.enter_context(tc.tile_pool(name="wp", bufs=1))
        ypool = ctx.enter_context(tc.tile_pool(name="yp", bufs=8))
        opool = ctx.enter_context(tc.tile_pool(name="op", bufs=1))
        pspool = ctx.enter_context(tc.tile_pool(name="ps", bufs=1,
                                                space="PSUM"))

        wtile = wpool.tile([P, NLAG, MC], F16)
        psums = [pspool.tile([MC, NB], F32, name=f"ps{t}")
                 for t in range(NBT)]
        warm = pspool.tile([MC, NB], F32, name="warm")
        outt = opool.tile([MC, BS], F16)

        # y chunk lag-ranges: two small 2-lag chunks first (PE starts after
        # only w0 + 0.5 MB of y), then 4-lag chunks.
        CH = [(0, 2), (2, 4)] + [(4 + KJ * i, 4 + KJ * (i + 1))
                                 for i in range((NLAG - 4) // KJ)]
        ypool0 = ctx.enter_context(tc.tile_pool(name="yp0", bufs=2))
        ytiles = {}

        def issue_y(ci):
            lo, hi = CH[ci]
            pool = ypool0 if hi - lo < KJ else ypool
            yti = pool.tile([P, hi - lo, BS], F8)
            nc.sync.dma_start(yti[:], yt[:, lo:hi, :])
            ytiles[ci] = yti

        # issue order: w0, y0, y1, w1, y2, w2, w3, y3, w4..w7, rest lazily
        nc.sync.dma_start(wtile[:, 0:WL, :], w[:, 0:WL, :])
        issue_y(0)
        issue_y(1)
        nc.sync.dma_start(wtile[:, WL:2 * WL, :], w[:, WL:2 * WL, :])
        issue_y(2)
        for c in range(2, WCH):
            nc.sync.dma_start(wtile[:, c * WL:(c + 1) * WL, :],
                              w[:, c * WL:(c + 1) * WL, :])
            if c == 2:
                issue_y(3)

        # HAM ramp: PE activity as soon as w chunk 0 lands
        for wi in range(4):
            nc.tensor.matmul(warm[:], wtile[:, wi, :], wtile[:, 4:8, :],
                             start=(wi == 0), stop=(wi == 3))

        for ci in range(len(CH)):
            if ci not in ytiles:
                issue_y(ci)
            yti = ytiles.pop(ci)
            lo, hi = CH[ci]
            for jj in range(hi - lo):
                j = lo + jj
                lhsT = wtile[:, j, :]
                for t in range(NBT):
                    nc.tensor.matmul(psums[t][:], lhsT,
                                     yti[:, jj, ts(t, NB)],
                                     start=(j == 0), stop=(j == NLAG - 1))

        # evacuate: tiles 0,1 on DVE, tiles 2,3 on ACT (parallel), then
        # two half out-DMAs, each waiting a single engine's sem.
        nc.vector.tensor_copy(outt[:, ts(0, NB)], psums[0][:])
        nc.scalar.copy(outt[:, ts(2, NB)], psums[2][:])
        nc.vector.tensor_copy(outt[:, ts(1, NB)], psums[1][:])
        nc.scalar.copy(outt[:, ts(3, NB)], psums[3][:])
        nc.sync.dma_start(out[:, 0:2 * NB], outt[:, 0:2 * NB])
        nc.sync.dma_start(out[:, 2 * NB:], outt[:, 2 * NB:])

    return nc


def _strip_redundant_waits(nc):
    """Drop semaphore waits that are provably implied by other waits.

    Tile's add_semaphores pass is per-processor minimal but not transitively
    minimal; walrus codegen allows only one sync wait per DMA/Matmult/Drain
    HW instruction.  Model:
      - expand(s >= v) = {s >= v} union C[sat(s, v)] where sat is the
        instruction whose semaphore update first reaches v (updates on one
        engine sem / one DMA lane fire in order).
      - C[i] ("true once i's updates fired") = own updates + expand(own
        waits) + dispatch-knowledge (expand of same-engine predecessors'
        waits; sequencers evaluate waits in program order) + C[predecessor]
        chained in completion order: same engine for compute engines, same
        DMA lane for DMACopy (async transfers complete in ring order).
    A wait w on i is droppable iff w is in (expand of same-engine
    predecessors' waits) union (expand of i's other waits).
    """
    from concourse import mybir

    f = nc.m.functions[0]
    insts = [i for blk in f.blocks for i in blk.instructions]

    def waits(i):
        si = i.sync_info
        return [(w.ant_name, w.wait_value) for w in (si.on_wait or [])] \
            if si else []

    def updates(i):
        si = i.sync_info
        return list(si.on_update or []) if si else []

    by_engine = {}
    for i in insts:
        by_engine.setdefault(str(i.engine), []).append(i)

    COMPUTE = {"EngineType.PE", "EngineType.DVE", "EngineType.Activation",
               "EngineType.Pool"}

    # cumulative update values per sem, in program order of the updater
    sem_updates = {}           # sem -> [(inst_name, cumulative)]
    upd_of = {i.name: [] for i in insts}
    for eng, lst in by_engine.items():
        for i in lst:
            for u in updates(i):
                cum = sem_updates.setdefault(u.ant_name, [])
                prev = cum[-1][1] if cum else 0
                val = u.update_value if u.update_mode == "sem-add-imm" else 1
                cum.append((i.name, prev + val))
                upd_of[i.name].append((u.ant_name, prev + val))

    def satisfier(sem, v):
        for name, val in sem_updates.get(sem, ()):
            if val >= v:
                return name
        return None

    # completion-order predecessor: same engine (compute) or same DMA lane
    comp_pred = {}
    last_on_lane = {}
    for eng, lst in by_engine.items():
        prev = None
        for i in lst:
            if type(i).__name__ == "InstDMACopy":
                lanes = [s for s, _ in upd_of[i.name]]
                lane = lanes[0] if lanes else None
                comp_pred[i.name] = last_on_lane.get(lane)
                if lane is not None:
                    last_on_lane[lane] = i.name
            elif eng in COMPUTE:
                comp_pred[i.name] = prev
            else:
                comp_pred[i.name] = None
            prev = i.name

    # dispatch-order predecessor (same engine, any type)
    disp_pred = {}
    for eng, lst in by_engine.items():
        prev = None
        for i in lst:
            disp_pred[i.name] = prev
            prev = i.name

    # Pre-pass: drop a DMA's wait on its OWN lane sem when satisfied by an
    # earlier DMA on that lane — vacuous under the per-lane ring-order
    # model comp_pred already assumes (completions fire in issue order, so
    # this DMA's sem updates cannot overtake the predecessor's).  Must run
    # BEFORE the DW/C fixpoint so dispatch knowledge never cites a dropped
    # wait.
    for i in insts:
        si = i.sync_info
        if not si or not si.on_wait or type(i).__name__ != "InstDMACopy":
            continue
        own = {s for s, _ in upd_of[i.name]}
        keep = []
        for w in si.on_wait:
            drop = False
            if w.ant_name in own:
                j = satisfier(w.ant_name, w.wait_value)
                p = comp_pred.get(i.name)
                while p is not None:
                    if p == j:
                        drop = True
                        break
                    p = comp_pred.get(p)
            if not drop:
                keep.append(w)
        if len(keep) != len(si.on_wait):
            i.sync_info = mybir.SyncInfo(
                on_wait=keep, on_update=list(si.on_update or []))

    C = {i.name: {} for i in insts}
    DW = {i.name: {} for i in insts}   # dispatch knowledge (pred waits, cum.)
    name2inst = {i.name: i for i in insts}

    def merge(dst, src_items):
        ch = False
        for s, v in src_items:
            if dst.get(s, 0) < v:
                dst[s] = v
                ch = True
        return ch

    changed = True
    rounds = 0
    while changed and rounds < 100:
        changed = False
        rounds += 1
        for i in insts:
            n = i.name
            # DW: dispatch knowledge = pred's DW + expand(pred's waits)
            dp = disp_pred[n]
            if dp is not None:
                changed |= merge(DW[n], DW[dp].items())
                for s, v in waits(name2inst[dp]):
                    changed |= merge(DW[n], [(s, v)])
                    j = satisfier(s, v)
                    if j is not None:
                        changed |= merge(DW[n], C[j].items())
            # C: completion closure
            changed |= merge(C[n], DW[n].items())
            changed |= merge(C[n], upd_of[n])
            cp = comp_pred.get(n)
            if cp is not None:
                changed |= merge(C[n], C[cp].items())
            for s, v in waits(i):
                changed |= merge(C[n], [(s, v)])
                j = satisfier(s, v)
                if j is not None:
                    changed |= merge(C[n], C[j].items())

    for i in insts:
        si = i.sync_info
        if not si or len(si.on_wait or []) <= 1:
            continue
        if type(i).__name__ not in ("InstDMACopy", "InstMatmult",
                                    "InstDrain"):
            continue
        keep = []
        for w in si.on_wait:
            avail = dict(DW[i.name])
            for w2 in si.on_wait:
                if w2 is w:
                    continue
                merge(avail, [(w2.ant_name, w2.wait_value)])
                j = satisfier(w2.ant_name, w2.wait_value)
                if j is not None:
                    merge(avail, C[j].items())
            if avail.get(w.ant_name, 0) < w.wait_value:
                keep.append(w)
        if len(keep) > 1 and type(i).__name__ == "InstDrain":
            # A drain is pure synchronization: hoist surplus waits onto
            # sequencer NoOps right before it (sequencer ops are exempt
            # from the walrus 1-wait limit and evaluate in program order).
            blk = next(b for b in f.blocks if i in b.instructions)
            pos = list(blk.instructions).index(i)
            nops = []
            for k, w in enumerate(keep[:-1]):
                nop = mybir.InstNoOp(name=f"{i.name}-hw{k}", ins=[], outs=[])
                nop.engine = i.engine
                nop.sync_info = mybir.SyncInfo(on_wait=[w], on_update=[])
                nops.append(nop)
            blk.instructions = (list(blk.instructions)[:pos] + nops
                                + list(blk.instructions)[pos:])
            keep = keep[-1:]
        if len(keep) > 1:
            raise RuntimeError(
                f"{i.name}: still {len(keep)} waits after stripping: "
                f"{[(w.ant_name, w.wait_value) for w in keep]}")
        if len(keep) != len(si.on_wait):
            i.sync_info = mybir.SyncInfo(
                on_wait=keep, on_update=list(si.on_update or []))
    return nc


def _get_nc(mode):
    if mode not in _cached_nc:
        build = _build_nc_f8 if mode == "f8a" else (lambda: _build_nc(mode))
        _cached_nc[mode] = _strip_redundant_waits(build())
    return _cached_nc[mode]


def kernel(y_hist, M_bar_0, M_bar, M_0l, M_il, sigma_powered, phi,
           lambda_powered, varphi):
    from concourse.bass_utils import run_bass_kernel_spmd

    mp = _mode_params(MODE)
    y_hist = np.ascontiguousarray(np.asarray(y_hist, dtype=np.float32))
    w_dev = _fold_weights(np.asarray(M_bar_0), np.asarray(M_bar),
                          np.asarray(M_0l), np.asarray(M_il),
                          np.asarray(sigma_powered), np.asarray(phi),
                          np.asarray(lambda_powered), np.asarray(varphi))
    if mp["yscale"] is not None:
        w_dev = w_dev / np.float32(mp["yscale"])
    if mp["wnpdt"] != np.float32:
        w_dev = w_dev.astype(mp["wnpdt"])
    if MODE == "f8a":
        shards = _transpose_shards_pjb(y_hist, mp["npdt"], mp["yscale"])
    else:
        shards = _transpose_shards(y_hist, mp["npdt"], mp["yscale"])

    nc = _get_nc(MODE)
    in_maps = [{"yt": shards[ci], "w": w_dev} for ci in range(N_CORES)]
    res = run_bass_kernel_spmd(nc, in_maps, list(range(N_CORES)), trace=TRACE)

    if TRACE:
        kernel.last_result = res

    out = np.empty((B, MC), np.float32)
    for ci in range(N_CORES):
        out[ci * BS:(ci + 1) * BS] = res.results[ci]["out"].T
    return out

